# revision 42
# baseline (speedup 1.0000x reference)
"""Trainium2 Bass kernel for nn_Block_46471546143558 (moe_routing).

Transformer block: h = x + Attn(LN1(x)); y = h + MoE(LN2(h)).
B=2, T=2048, D=1024, H=16 heads (hd=64), E=8 experts, top-2, cap=640.

Sharding over 8 NeuronCores:
  - attention: head-parallel (2 heads/core); LN/proj/residual token-parallel
    (each core owns a contiguous 512-token shard of the flattened 4096)
  - MoE: expert-parallel (1 expert/core); per-token top-2 masks are computed
    once on the owner core and all-gathered fused with moe_in
  - collectives: AG(x_ln) -> A2A(attn heads->token shards) ->
    AG(moe_in|mask) -> AG(expert outputs). Output assembled shard-wise on
    host.

All math on device except weight preprocessing (transpose/slice/cast,
folding ln2_g/ln2_b into W1/b1) and constant tables (RoPE trig, causal
masks, iota/triangular matrices), which depend only on weights/shapes.
"""

import math
import sys

sys.path.insert(0, "/opt/trn_rl_repo")

import ml_dtypes
import numpy as np

import concourse.bass as bass
import concourse.mybir as mybir
import concourse.tile as tile
from concourse import bacc
from concourse.bass import IndirectOffsetOnAxis
from concourse.bass_utils import run_bass_kernel_spmd

B, T, D, H, E, K = 2, 2048, 1024, 16, 8, 2
HD = D // H  # 64
N = B * T  # 4096
CAP = math.ceil(1.25 * N / E)  # 640
NC = 8
SH = N // NC  # 512
NT = N // 128  # 32
ST = CAP // 128  # 5
F1 = 4 * D  # 4096
MW = 1032  # moe_in(1024) + mask(8) fused AG payload width

FP = mybir.dt.float32
BF = mybir.dt.bfloat16
I32 = mybir.dt.int32
ATT = mybir.dt.float16  # attention pipeline dtype (fp16: 11-bit mantissa
# keeps gate-logit error far below the top-2 decision margins)
NPATT = np.float16

OOB = 1.0e9
AF = mybir.ActivationFunctionType
AL = mybir.AluOpType


def build(debug=False):
    nc = bacc.Bacc("TRN2", target_bir_lowering=False, debug=False,
                   enable_asserts=True, num_devices=NC)

    def din(name, shape, dt):
        return nc.dram_tensor(name, list(shape), dt, kind="ExternalInput").ap()

    xT = din("xT", [128, 8, SH], FP)
    qk_w = din("qk_w", [128, 8, 256], ATT)
    v_w = din("v_w", [128, 8, 128], ATT)
    proj_w = din("proj_w", [128, 8, 1024], ATT)
    gate_w = din("gate_w", [128, 8, 8], FP)
    Cqk = din("Cqk", [128, NT, 128], ATT)
    Sqk = din("Sqk", [128, NT, 128], ATT)
    identb = din("identb", [128, 128], BF)
    identf = din("identf", [128, 128], FP)
    tri = din("tri", [128, 128], BF)
    tri16 = din("tri16", [128, 128], ATT)
    m00 = din("m00", [128, 128], BF)
    m01 = din("m01", [128, 128], BF)
    gpc = din("gpc", [64, 2, 8], FP)
    onesf = din("onesf", [128, 1], FP)
    onesb = din("onesb", [128, 1], BF)
    ones_r_f = din("ones_r_f", [1, 128], FP)
    ln1g = din("ln1g", [128, 8], FP)
    ln1b = din("ln1b", [128, 8], FP)
    cc1 = din("cc1", [128, 8], FP)
    cc2 = din("cc2", [128, 8], FP)
    base8 = din("base8", [128, 8], FP)
    esel = din("esel", [128, 8], FP)
    shsel = din("shsel", [128, 4, NT], FP)
    tid1 = din("tid1", [128, NT], I32)
    w1l = din("w1l", [1024, F1], BF)
    b1l = din("b1l", [128, 32], FP)
    w2l = din("w2l", [F1, 1024], BF)
    b2l = din("b2l", [128, 1024], FP)

    y_out = nc.dram_tensor("y", [SH, D], FP, kind="ExternalOutput").ap()
    dbg = {}
    if debug:
        def dout(name, shape, dt=FP):
            return nc.dram_tensor(name, list(shape), dt, kind="ExternalOutput").ap()
        dbg["xln"] = dout("dbg_xln", [128, 8, SH])
        dbg["attn"] = dout("dbg_attn", [128, N], ATT)
        dbg["h"] = dout("dbg_h", [128, 4, D])
        dbg["logits"] = dout("dbg_logits", [128, 4, 8])
        dbg["pos"] = dout("dbg_pos", [128, NT, 8])
        dbg["gidx"] = dout("dbg_gidx", [128, ST], I32)
        dbg["cidx"] = dout("dbg_cidx", [128, 4, 2], I32)
        dbg["g12"] = dout("dbg_g12", [128, 4, 2])
        dbg["eo"] = dout("dbg_eo", [CAP, D], BF)
        dbg["qT"] = dout("dbg_qT", [128, N], ATT)
        dbg["kT"] = dout("dbg_kT", [128, N], ATT)

    tbl8 = [nc.dram_tensor(f"tbl{i}", [768, 1], I32, kind="Internal").ap()
            for i in range(8)]
    a2a_out = nc.dram_tensor("a2ao", [NC * 128, 512], ATT, kind="Internal").ap()
    a2alg_out = nc.dram_tensor("a2alo", [NC * 16, 512], BF,
                               kind="Internal").ap()
    ag3m_out = nc.dram_tensor("ag3mo", [N, 8], BF, kind="Internal",
                              addr_space="Shared").ap()
    ag3_out = nc.dram_tensor("ag3o", [N, 1024], BF, kind="Internal",
                             addr_space="Shared").ap()
    ag4_out = nc.dram_tensor("ag4o", [NC * CAP, D], BF, kind="Internal",
                             addr_space="Shared").ap()
    RG = [list(range(NC))]

    with tile.TileContext(nc) as tc:
        with tc.tile_pool(name="dram", bufs=1, space="DRAM") as dram, \
             tc.tile_pool(name="pers", bufs=1) as pers, \
             tc.tile_pool(name="const", bufs=1) as const:
            ag1_in = dram.tile([128, 8, SH], ATT)
            ag1_out = dram.tile([NC, 128, 8, SH], ATT, addr_space="Shared")
            a2a_in = dram.tile([NC * 128, 512], ATT)
            a2alg_in = dram.tile([NC * 16, 512], BF)
            ag3m_in = dram.tile([SH, 8], BF)
            ag3_in = dram.tile([SH, 1024], BF)
            ag4_in = dram.tile([CAP, D], BF)

            h_sb = pers.tile([128, 4, D], FP)
            m2c = pers.tile([128, 4], FP)
            r2c = pers.tile([128, 4], FP)
            lg_my = pers.tile([128, 4, 8], FP)
            g12 = pers.tile([128, 4, 2], FP)
            cidx = pers.tile([128, 4, 2], I32)
            gidx = pers.tile([128, ST], I32)
            maskm = pers.tile([128, 4, 8], BF)
            oh0b = pers.tile([128, 4, 8], FP)
            oh1b = pers.tile([128, 4, 8], FP)

            qkw_sb = const.tile([128, 8, 256], ATT)
            vw_sb = const.tile([128, 8, 128], ATT)
            identb_sb = const.tile([128, 128], BF)
            identf_sb = const.tile([128, 128], FP)
            tri_sb = const.tile([128, 128], BF)
            tri16_sb = const.tile([128, 128], ATT)
            m00_sb = const.tile([128, 128], BF)
            m01_sb = const.tile([128, 128], BF)
            gpc_sb = const.tile([64, 2, 8], FP)
            cqk_sb = const.tile([128, NT, 128], ATT)
            sqk_sb = const.tile([128, NT, 128], ATT)
            onesf_sb = const.tile([128, 1], FP)
            onesb_sb = const.tile([128, 1], BF)
            orf_sb = const.tile([1, 128], FP)
            ln1g_sb = const.tile([128, 8], FP)
            ln1b_sb = const.tile([128, 8], FP)
            cc1_sb = const.tile([128, 8], FP)
            cc2_sb = const.tile([128, 8], FP)
            base8_sb = const.tile([128, 8], FP)
            esel_sb = const.tile([128, 8], FP)
            shsel_sb = const.tile([128, 4, NT], FP)
            tid1_sb = const.tile([128, NT], I32)
            gatew_sb = const.tile([128, 8, 8], FP)
            b1l_sb = const.tile([128, 32], FP)
            b2l_sb = const.tile([128, 1024], FP)
            eps_sb = const.tile([128, 1], FP)
            nc.vector.memset(eps_sb[:], 1e-5)
            with tc.tile_pool(name="attn", bufs=1) as attn:
                qh0 = attn.tile([64, N], ATT)
                qh1 = attn.tile([64, N], ATT)
                kh0 = attn.tile([64, N], ATT)
                kh1 = attn.tile([64, N], ATT)
                v_tm = attn.tile([128, NT, 256], ATT)
                aT0 = attn.tile([128, N], ATT)
                aT1 = attn.tile([128, N], ATT)
                xT_sb = attn.tile([128, 8, SH], FP)
                v12all = attn.tile([128, 4, 2], FP)
                for sb, dr in [(xT_sb, xT),
                               (qkw_sb, qk_w), (vw_sb, v_w), (identb_sb, identb),
                               (identf_sb, identf), (tri_sb, tri), (m00_sb, m00),
                               (tri16_sb, tri16),
                               (m01_sb, m01), (gpc_sb, gpc), (onesf_sb, onesf),
                               (onesb_sb, onesb), (orf_sb, ones_r_f),
                               (ln1g_sb, ln1g), (ln1b_sb, ln1b),
                               (cc1_sb, cc1), (cc2_sb, cc2), (base8_sb, base8),
                               (esel_sb, esel), (shsel_sb, shsel), (tid1_sb, tid1),
                               (gatew_sb, gate_w), (b1l_sb, b1l), (b2l_sb, b2l),
                               (cqk_sb, Cqk), (sqk_sb, Sqk)]:
                    nc.sync.dma_start(sb[:], dr[:])

                # ======== P1: LN1 (feature-major) + AG1 ========
                with tc.tile_pool(name="p1", bufs=1) as p1, \
                     tc.tile_pool(name="p1ps", bufs=2, space="PSUM") as p1ps:
                    xTb = p1.tile([128, 8, SH], BF)
                    sqb = p1.tile([128, 8, SH], BF)
                    for o in range(8):
                        nc.vector.tensor_copy(xTb[:, o], xT_sb[:, o])
                        nc.vector.tensor_mul(sqb[:, o], xTb[:, o], xTb[:, o])
                    msum = p1ps.tile([1, SH], FP, tag="ms")
                    ssum = p1ps.tile([1, SH], FP, tag="ss")
                    for o in range(8):
                        nc.tensor.matmul(msum[:], onesb_sb[:], xTb[:, o],
                                         start=(o == 0), stop=(o == 7))
                    for o in range(8):
                        nc.tensor.matmul(ssum[:], onesb_sb[:], sqb[:, o],
                                         start=(o == 0), stop=(o == 7))
                    mrow = p1.tile([1, SH], FP)
                    rrow = p1.tile([1, SH], FP)
                    nc.scalar.mul(mrow[:], msum[:], 1.0 / D)
                    nc.scalar.mul(rrow[:], ssum[:], 1.0 / D)
                    m2row = p1.tile([1, SH], FP)
                    nc.vector.tensor_mul(m2row[:], mrow[:], mrow[:])
                    nc.vector.tensor_sub(rrow[:], rrow[:], m2row[:])
                    nc.scalar.activation(rrow[:], rrow[:], AF.Sqrt,
                                         bias=eps_sb[0:1, 0:1])
                    nc.vector.reciprocal_approx_fast(rrow[:], rrow[:])
                    mb = p1ps.tile([128, SH], FP, tag="bc")
                    rb = p1ps.tile([128, SH], FP, tag="bc")
                    nc.tensor.matmul(mb[:], orf_sb[:], mrow[:], start=True, stop=True)
                    nc.tensor.matmul(rb[:], orf_sb[:], rrow[:], start=True, stop=True)
                    xln = p1.tile([128, 8, SH], ATT)
                    for o in range(8):
                        t1 = p1.tile([128, SH], FP, tag="t1")
                        nc.vector.tensor_sub(t1[:], xT_sb[:, o], mb[:])
                        nc.vector.tensor_mul(t1[:], t1[:], rb[:])
                        nc.vector.tensor_scalar(xln[:, o], t1[:],
                                                ln1g_sb[:, o:o + 1], ln1b_sb[:, o:o + 1],
                                                AL.mult, AL.add)
                    nc.sync.dma_start(ag1_in[:], xln[:])
                    if debug:
                        xlnf = p1.tile([128, 8, SH], FP)
                        nc.vector.tensor_copy(xlnf[:], xln[:])
                        nc.sync.dma_start(dbg["xln"][:], xlnf[:])
                nc.gpsimd.collective_compute("AllGather", AL.bypass, RG,
                                             ins=[ag1_in.opt()], outs=[ag1_out.opt()])

                # ======== P2: QKV (fused q|k) + RoPE (bf16) + V token-major ========
                with tc.tile_pool(name="p2", bufs=2) as p2, \
                     tc.tile_pool(name="p2ps", bufs=2, space="PSUM") as p2ps, \
                     tc.tile_pool(name="p2v", bufs=2, space="PSUM") as p2v:
                    nc.vector.memset(v_tm[:, :, 0:64], 0.0)
                    nc.vector.memset(v_tm[:, :, 128:192], 0.0)
                    nc.vector.memset(v_tm[:, :, 0:1], 1.0)
                    nc.vector.memset(v_tm[:, :, 128:129], 1.0)
                    v_tm4 = v_tm.rearrange("p t (a b) -> p t a b", a=2)
                    for s in range(NC):
                        xl = p2.tile([128, 8, SH], ATT, tag="xl")
                        nc.sync.dma_start(xl[:], ag1_out[s])
                        Cv = cqk_sb[:, 4 * s:4 * s + 4].rearrange("p t f -> p (t f)")
                        Sv = sqk_sb[:, 4 * s:4 * s + 4].rearrange("p t f -> p (t f)")
                        for mt in range(2):
                            qdst = qh0 if mt == 0 else qh1
                            kdst = kh0 if mt == 0 else kh1
                            ps = p2ps.tile([128, SH], FP, tag=f"qk{mt}", bufs=2)
                            for o in range(8):
                                nc.tensor.matmul(ps[:], qkw_sb[:, o, 128 * mt:128 * mt + 128],
                                                 xl[:, o], start=(o == 0), stop=(o == 7))
                            psb = p2.tile([128, SH], ATT, tag="psb")
                            nc.scalar.copy(psb[:], ps[:])
                            tmp = p2.tile([128, SH], ATT, tag="tmp")
                            acc = p2.tile([128, SH], ATT, tag="acc")
                            nc.vector.tensor_mul(tmp[0:32], psb[32:64], Sv[32:64])
                            nc.vector.tensor_mul(tmp[32:64], psb[0:32], Sv[0:32])
                            nc.vector.tensor_mul(tmp[64:96], psb[96:128], Sv[96:128])
                            nc.vector.tensor_mul(tmp[96:128], psb[64:96], Sv[64:96])
                            nc.vector.tensor_mul(acc[:], psb[:], Cv[:])
                            nc.vector.tensor_add(qdst[:, SH * s:SH * (s + 1)],
                                                 acc[0:64], tmp[0:64])
                            nc.vector.tensor_add(kdst[:, SH * s:SH * (s + 1)],
                                                 acc[64:128], tmp[64:128])
                        for tt in range(4):
                            vps = p2v.tile([128, 128], FP, tag="v")
                            for o in range(8):
                                nc.tensor.matmul(vps[:], xl[:, o, 128 * tt:128 * tt + 128],
                                                 vw_sb[:, o], start=(o == 0), stop=(o == 7))
                            g = 4 * s + tt
                            nc.vector.tensor_copy(
                                v_tm4[:, g, :, 64:128],
                                vps.rearrange("p (a b) -> p a b", a=2))

                if debug:
                    nc.sync.dma_start(dbg["qT"][0:64, :], qh0[:])
                    nc.sync.dma_start(dbg["qT"][64:128, :], qh1[:])
                    nc.sync.dma_start(dbg["kT"][0:64, :], kh0[:])
                    nc.sync.dma_start(dbg["kT"][64:128, :], kh1[:])
                # ======== P3: scores + softmax + PV + normalize; A2A ========
                lgacc = attn.tile([128, 8, 4, 8], FP)
                with tc.tile_pool(name="p3", bufs=3) as p3, \
                     tc.tile_pool(name="p3ps", bufs=2, space="PSUM") as p3ps, \
                     tc.tile_pool(name="p3po", bufs=2, space="PSUM") as p3po:
                    for hh in range(2):
                        aT = aT0 if hh == 0 else aT1
                        qh = qh0 if hh == 0 else qh1
                        kh = kh0 if hh == 0 else kh1
                        for qb in range(8):
                            seq, qbl = qb // 4, qb % 4
                            kts = list(range(16 * seq, 16 * seq + 4 * qbl + 4))
                            nk = len(kts)
                            ov = p3po.tile([128, 512], FP, tag="ov")
                            for i, kt in enumerate(kts):
                                loc = kt - 16 * seq - 4 * qbl
                                c0 = 128 * loc if loc > 0 else 0
                                sc = p3ps.tile([128, 512], FP, tag="sc")
                                nc.tensor.matmul(sc[:, c0:512],
                                                 kh[:, 128 * kt:128 * kt + 128],
                                                 qh[:, 512 * qb + c0:512 * (qb + 1)],
                                                 start=True, stop=True)
                                ex = p3.tile([128, 512], ATT, tag="ex")
                                nc.scalar.activation(ex[:, c0:512], sc[:, c0:512], AF.Exp)
                                if loc >= 0:
                                    nc.vector.tensor_mul(ex[:, c0:c0 + 128],
                                                         ex[:, c0:c0 + 128], tri16_sb[:])
                                nc.tensor.matmul(ov[:, c0:512],
                                                 v_tm[:, kt, 128 * hh:128 * hh + 128],
                                                 ex[:, c0:512], start=(i == 0),
                                                 stop=(i == nk - 1),
                                                 skip_group_check=True)
                            rrow = p3.tile([1, 512], FP, tag="den")
                            nc.vector.reciprocal_approx_fast(rrow[:], ov[0:1])
                            rbp = p3po.tile([128, 512], FP, tag="rb", bufs=2)
                            nc.tensor.matmul(rbp[:], orf_sb[:], rrow[:],
                                             start=True, stop=True)
                            rbs = p3.tile([128, 512], FP, tag="rbs")
                            nc.scalar.copy(rbs[64:128], rbp[64:128])
                            aTf = p3.tile([64, 512], FP, tag="atf")
                            nc.vector.tensor_mul(aTf[:], ov[64:128], rbs[64:128])
                            nc.vector.tensor_copy(
                                aT[64:128, 512 * qb:512 * (qb + 1)], aTf[:])
                            lq = p3po.tile([128, 4, 8], FP, tag="lq", bufs=2)
                            for j in range(4):
                                nc.tensor.matmul(lq[:, j],
                                                 aTf[:, 128 * j:128 * j + 128],
                                                 gpc_sb[:, hh], start=(j == 0),
                                                 stop=(j == 3))
                            if hh == 0:
                                nc.vector.tensor_copy(lgacc[:, qb], lq[:])
                            else:
                                nc.vector.tensor_add(lgacc[:, qb], lgacc[:, qb],
                                                     lq[:])
                    a2a_v = a2a_in.rearrange("(d p) t -> p d t", p=128)
                    nc.sync.dma_start(a2a_v[0:64],
                                      aT0[64:128].rearrange("p (d t) -> p d t", d=8))
                    nc.sync.dma_start(a2a_v[64:128],
                                      aT1[64:128].rearrange("p (d t) -> p d t", d=8))
                    # fp32 gate-logit partials as bf16 hi|lo pairs, token-major
                    lgts = p3.tile([128, 8, 4, 16], BF, tag="lgts")
                    hif = p3.tile([128, 8, 4, 8], FP, tag="hif")
                    nc.vector.tensor_copy(lgts[:, :, :, 0:8], lgacc[:])
                    nc.vector.tensor_copy(hif[:], lgts[:, :, :, 0:8])
                    nc.vector.tensor_sub(lgts[:, :, :, 8:16], lgacc[:], hif[:])
                    blobv = a2alg_in.rearrange("(d r) t -> d (r t)", r=16).rearrange(
                        "d (g p e) -> p d g e", p=128, e=16)
                    for dd in range(8):
                        nc.sync.dma_start(blobv[:, dd], lgts[:, dd])
                    if debug:
                        nc.sync.dma_start(dbg["attn"][0:64, :], aT0[64:128])
                        nc.sync.dma_start(dbg["attn"][64:128, :], aT1[64:128])
                nc.gpsimd.collective_compute("AllToAll", AL.bypass, RG,
                                             ins=[a2a_in.opt()], outs=[a2a_out.opt()])
                nc.gpsimd.collective_compute("AllToAll", AL.bypass, RG,
                                             ins=[a2alg_in.opt()],
                                             outs=[a2alg_out.opt()])

                # ======== P4: proj + h + LN2 + logits + mask + moe_in; AG3 ========
                with tc.tile_pool(name="p4", bufs=1) as p4, \
                     tc.tile_pool(name="p4ps", bufs=2, space="PSUM") as p4ps, \
                     tc.tile_pool(name="p4pt", bufs=2, space="PSUM") as p4pt:
                    # zero the 8 dispatch tables early (used in P5)
                    ztab = p4.tile([128, 6], I32, tag="ztab")
                    nc.vector.memset(ztab[:], 0)
                    for i in range(8):
                        nc.sync.dma_start(
                            tbl8[i].rearrange("(o p) f -> p o f", p=128),
                            ztab[:, :, None])
                    # gate logits, x-residual part (fp32; runs during A2A)
                    lgA_ps = p4pt.tile([8, SH], FP, tag="lps")
                    for ki in range(8):
                        nc.tensor.matmul(lgA_ps[:], gatew_sb[:, ki], xT_sb[:, ki],
                                         start=(ki == 0), stop=(ki == 7))
                    lgA_sb = p4.tile([8, SH], FP, tag="lsb")
                    nc.vector.tensor_copy(lgA_sb[:], lgA_ps[:])
                    myc = p4.tile([128, 8, 512], ATT)
                    nc.sync.dma_start(myc[:],
                                      a2a_out.rearrange("(s p) t -> p s t", p=128))
                    # gate logits, attention part: sum the 8 cores' hi|lo partials
                    lgr = p4.tile([128, 8, 4, 16], BF, tag="lgr")
                    lgbv = a2alg_out.rearrange("(s r) t -> s (r t)", r=16).rearrange(
                        "s (g p e) -> p s g e", p=128, e=16)
                    for ss in range(8):
                        nc.sync.dma_start(lgr[:, ss], lgbv[:, ss])
                    lgr2 = p4.tile([128, 8, 4, 8], FP, tag="lgr2")
                    nc.vector.tensor_tensor(lgr2[:], lgr[:, :, :, 0:8],
                                            lgr[:, :, :, 8:16], AL.add)
                    lgsum = p4.tile([128, 4, 8], FP, tag="lgsum")
                    nc.vector.tensor_reduce(
                        lgsum[:, :, :, None],
                        lgr2.rearrange("p s g e -> p g e s"),
                        axis=mybir.AxisListType.X, op=AL.add)
                    hT = p4.tile([128, 8, SH], FP)
                    pw = p4.tile([128, 8, 1024], ATT)
                    nc.sync.dma_start(pw[:], proj_w[:])
                    for do in range(8):
                        pp = p4ps.tile([128, SH], FP, tag="pp")
                        for ki in range(8):
                            nc.tensor.matmul(pp[:], pw[:, ki, 128 * do:128 * do + 128],
                                             myc[:, ki], start=(ki == 0), stop=(ki == 7))
                        nc.vector.tensor_add(hT[:, do], pp[:], xT_sb[:, do])
                        for tt in range(4):
                            tp = p4pt.tile([128, 128], FP, tag="tp")
                            nc.tensor.transpose(tp[:], hT[:, do, 128 * tt:128 * tt + 128],
                                                identf_sb[:])
                            nc.vector.tensor_copy(h_sb[:, tt, 128 * do:128 * do + 128], tp[:])
                    scr = p4.tile([128, D], FP, tag="scr")
                    sqc = p4.tile([128, 4], FP, tag="sqc")
                    for tt in range(4):
                        nc.vector.tensor_reduce(m2c[:, tt:tt + 1], h_sb[:, tt],
                                                axis=mybir.AxisListType.X, op=AL.add)
                        nc.scalar.activation(scr[:], h_sb[:, tt], AF.Square,
                                             accum_out=sqc[:, tt:tt + 1])
                    nc.vector.tensor_scalar_mul(m2c[:], m2c[:], 1.0 / D)
                    nc.vector.tensor_scalar_mul(sqc[:], sqc[:], 1.0 / D)
                    vv = p4.tile([128, 4], FP, tag="vv")
                    nc.vector.tensor_mul(vv[:], m2c[:], m2c[:])
                    nc.vector.tensor_sub(vv[:], sqc[:], vv[:])
                    nc.scalar.activation(vv[:], vv[:], AF.Sqrt, bias=eps_sb[:, 0:1])
                    nc.vector.reciprocal(r2c[:], vv[:])
                    rm = p4.tile([128, 4], FP, tag="rm")
                    nc.vector.tensor_mul(rm[:], r2c[:], m2c[:])
                    t8 = p4.tile([128, 8], FP, tag="t8")
                    lg8 = p4.tile([128, 8], FP, tag="lg8")
                    for tt in range(4):
                        ltp = p4pt.tile([128, 8], FP, tag="ltp")
                        nc.tensor.transpose(ltp[:], lgA_sb[0:8, 128 * tt:128 * tt + 128],
                                            identf_sb[0:8, 0:8])
                        nc.vector.tensor_add(lg8[:], ltp[:], lgsum[:, tt])
                        nc.vector.tensor_scalar_mul(lg_my[:, tt], lg8[:], r2c[:, tt:tt + 1])
                        nc.vector.tensor_scalar_mul(t8[:], cc1_sb[:], rm[:, tt:tt + 1])
                        nc.vector.tensor_sub(lg_my[:, tt], lg_my[:, tt], t8[:])
                        nc.vector.tensor_add(lg_my[:, tt], lg_my[:, tt], cc2_sb[:])
                    # top-2 mask per own token (once, on the owner core)
                    v8 = p4.tile([128, 8], FP, tag="v8")
                    for tt in range(4):
                        nc.vector.max(v8[:], lg_my[:, tt])
                        nc.vector.tensor_copy(v12all[:, tt], v8[:, 0:2])
                        nc.vector.tensor_scalar(maskm[:, tt], lg_my[:, tt],
                                                v8[:, 1:2], None, AL.is_ge)
                    # launch the tiny mask AllGather first: routing (P5) only
                    # needs masks, and can overlap the big moe_in AllGather
                    nc.sync.dma_start(
                        ag3m_in.rearrange("(t p) e -> p t e", p=128), maskm[:])
                    nc.gpsimd.collective_compute(
                        "AllGather", AL.bypass, RG,
                        ins=[ag3m_in.opt()], outs=[ag3m_out.opt()])
                    moein = p4.tile([128, 4, D], BF)
                    for tt in range(4):
                        nc.vector.tensor_scalar(moein[:, tt], h_sb[:, tt],
                                                m2c[:, tt:tt + 1], r2c[:, tt:tt + 1],
                                                AL.subtract, AL.mult)
                    nc.sync.dma_start(
                        ag3_in.rearrange("(t p) d -> p t d", p=128), moein[:])
                    if debug:
                        nc.sync.dma_start(dbg["h"][:], h_sb[:])
                        nc.sync.dma_start(dbg["logits"][:], lg_my[:])
                nc.gpsimd.collective_compute("AllGather", AL.bypass, RG,
                                             ins=[ag3_in.opt()], outs=[ag3_out.opt()])

                # overlapped with AG3: per-own-token gates (g12) and top1/top2 onehots
                with tc.tile_pool(name="p4b", bufs=2) as p4b:
                    ex8 = p4b.tile([128, 8], FP, tag="ex8")
                    den8 = p4b.tile([128, 1], FP, tag="den8")
                    for tt in range(4):
                        nc.scalar.activation(ex8[:], lg_my[:, tt], AF.Exp,
                                             accum_out=den8[:])
                        nc.vector.reciprocal(den8[:], den8[:])
                        nc.scalar.activation(g12[:, tt], v12all[:, tt], AF.Exp)
                        nc.vector.tensor_scalar_mul(g12[:, tt], g12[:, tt], den8[:])
                        nc.vector.tensor_scalar(oh0b[:, tt], lg_my[:, tt],
                                                v12all[:, tt, 0:1], None, AL.is_ge)
                        nc.vector.tensor_scalar(oh1b[:, tt], lg_my[:, tt],
                                                v12all[:, tt, 1:2], None, AL.is_ge)
                    nc.vector.tensor_sub(oh1b[:], oh1b[:], oh0b[:])

            # ======== P5: routing from gathered masks ========
            with tc.tile_pool(name="p5", bufs=2) as p5, \
                 tc.tile_pool(name="p5ps", bufs=2, space="PSUM") as p5ps:
                mask = p5.tile([128, NT, 8], BF)
                nc.sync.dma_start(
                    mask[:], ag3m_out.rearrange("(g p) e -> p g e", p=128))
                # intra-group inclusive cumsum over each group's 128 tokens
                pos_ps = p5ps.tile([128, NT * 8], FP, tag="pos", bufs=1)
                nc.tensor.matmul(pos_ps[:], tri_sb[:],
                                 mask.rearrange("p t e -> p (t e)"),
                                 start=True, stop=True)
                pintra = p5.tile([128, NT, 8], FP)
                nc.vector.tensor_copy(pintra[:],
                                      pos_ps.rearrange("p (t e) -> p t e", e=8))
                pintb = p5.tile([128, NT, 8], BF, tag="pintb")
                nc.vector.tensor_copy(pintb[:], pintra[:])
                # per-(group,expert) totals onto partitions via PE transpose
                totc = p5.tile([128, 2], BF, tag="totc")
                pvb = pintb.rearrange("p t e -> p (t e)")
                for a in range(2):
                    tta = p5ps.tile([128, 128], BF, tag="tta")
                    nc.tensor.transpose(tta[:], pvb[:, 128 * a:128 * a + 128],
                                        identb_sb[:])
                    nc.vector.tensor_copy(totc[:, a:a + 1], tta[:, 127:128])
                # exclusive inter-group prefix per (g,e) partition
                pref_ps = p5ps.tile([128, 2], FP, tag="pref", bufs=1)
                nc.tensor.matmul(pref_ps[:, 0:1], m00_sb[:], totc[:, 0:1],
                                 start=True, stop=True)
                nc.tensor.matmul(pref_ps[:, 1:2], m01_sb[:], totc[:, 0:1],
                                 start=True, stop=False)
                nc.tensor.matmul(pref_ps[:, 1:2], m00_sb[:], totc[:, 1:2],
                                 start=False, stop=True)
                pref = p5.tile([128, 2], FP, tag="prefs")
                nc.vector.tensor_copy(pref[:], pref_ps[:])
                posb_ps = p5ps.tile([128, NT * 8], FP, tag="posb", bufs=1)
                for a in range(2):
                    prow_ps = p5ps.tile([1, 128], FP, tag="prow", bufs=1)
                    nc.tensor.transpose(prow_ps[:], pref[:, a:a + 1], identf_sb[:])
                    prow = p5.tile([1, 128], FP, tag=f"prow{a}")
                    nc.vector.tensor_copy(prow[:], prow_ps[:])
                    nc.tensor.matmul(posb_ps[:, 128 * a:128 * a + 128],
                                     orf_sb[:], prow[:], start=True, stop=True)
                pos = p5.tile([128, NT, 8], FP)
                nc.vector.tensor_tensor(pos[:], pintra[:],
                                        posb_ps.rearrange("p (t e) -> p t e", e=8),
                                        AL.add)
                if debug:
                    nc.sync.dma_start(dbg["pos"][:], pos[:])
                # dispatch offsets for my expert (batched over groups)
                esel_bc = esel_sb[:, None, :].to_broadcast([128, NT, 8])
                t328 = p5.tile([128, NT, 8], FP, tag="t328")
                pe = p5.tile([128, NT], FP, tag="pe")
                me = p5.tile([128, NT], FP, tag="me")
                nc.vector.tensor_tensor(t328[:], pos[:], esel_bc, AL.mult)
                nc.vector.tensor_reduce(pe[:, :, None], t328[:],
                                        axis=mybir.AxisListType.X, op=AL.add)
                nc.vector.tensor_tensor(t328[:], mask[:], esel_bc, AL.mult)
                nc.vector.tensor_reduce(me[:, :, None], t328[:],
                                        axis=mybir.AxisListType.X, op=AL.add)
                offf = p5.tile([128, NT], FP, tag="offf")
                nc.vector.tensor_scalar(offf[:], me[:], -OOB, OOB, AL.mult, AL.add)
                nc.vector.tensor_add(offf[:], offf[:], pe[:])
                nc.vector.tensor_scalar_add(offf[:], offf[:], -1.0)
                offi = p5.tile([128, NT], I32, tag="offi")
                nc.vector.tensor_copy(offi[:], offf[:])
                # scatter into 8 disjoint tables (no writer-writer serialization);
                # slots are unique across groups, so a sum-merge recovers the table
                for g in range(NT):
                    nc.gpsimd.indirect_dma_start(
                        out=tbl8[g % 8][:],
                        out_offset=IndirectOffsetOnAxis(ap=offi[:, g:g + 1],
                                                        axis=0),
                        in_=tid1_sb[:, g:g + 1],
                        in_offset=None, bounds_check=CAP - 1, oob_is_err=False)
                tread8 = p5.tile([128, 6, 8], I32, tag="tread8")
                for i in range(8):
                    nc.sync.dma_start(tread8[:, :, i:i + 1],
                                      tbl8[i].rearrange("(o p) f -> p o f", p=128))
                treadf = p5.tile([128, 6, 8], FP, tag="treadf")
                nc.vector.tensor_copy(treadf[:], tread8[:])
                trsum = p5.tile([128, 6], FP, tag="trsum")
                nc.vector.tensor_reduce(trsum[:, :, None], treadf[:],
                                        axis=mybir.AxisListType.X, op=AL.add)
                tread = p5.tile([128, 6], I32, tag="tread")
                nc.vector.tensor_copy(tread[:], trsum[:])
                nc.vector.tensor_scalar_add(gidx[:], tread[:, 0:ST], -1)
                if debug:
                    nc.sync.dma_start(dbg["gidx"][:], gidx[:])
                # my combine indices from pos + precomputed onehots
                myp = p5.tile([128, 4, 8], FP, tag="myp")
                tmpb = p5.tile([128, NT, 8], FP, tag="tmpb")
                for tt in range(4):
                    nc.vector.tensor_tensor(tmpb[:], pos[:],
                                            shsel_sb[:, tt, :, None].to_broadcast(
                                                [128, NT, 8]), AL.mult)
                    nc.vector.tensor_reduce(myp[:, tt, :, None],
                                            tmpb.rearrange("p t e -> p e t"),
                                            axis=mybir.AxisListType.X, op=AL.add)
                fb = p5.tile([128, 4, 8], FP, tag="fb")
                nc.vector.tensor_tensor(fb[:], myp[:],
                                        base8_sb[:, None, :].to_broadcast([128, 4, 8]),
                                        AL.add)
                t48 = p5.tile([128, 4, 8], FP, tag="t48")
                sl = p5.tile([128, 4], FP, tag="sl")
                fl = p5.tile([128, 4], FP, tag="fl")
                kf = p5.tile([128, 4], FP, tag="kf")
                tof = p5.tile([128, 4], FP, tag="tof")
                cidf = p5.tile([128, 4, 2], FP, tag="cidf")
                for kk, oh in ((0, oh0b), (1, oh1b)):
                    nc.vector.tensor_tensor(t48[:], myp[:], oh[:], AL.mult)
                    nc.vector.tensor_reduce(sl[:, :, None], t48[:],
                                            axis=mybir.AxisListType.X, op=AL.add)
                    nc.vector.tensor_tensor(t48[:], fb[:], oh[:], AL.mult)
                    nc.vector.tensor_reduce(fl[:, :, None], t48[:],
                                            axis=mybir.AxisListType.X, op=AL.add)
                    nc.vector.tensor_scalar(kf[:], sl[:], CAP + 0.5, None, AL.is_le)
                    nc.vector.tensor_scalar(tof[:], kf[:], -OOB, OOB, AL.mult, AL.add)
                    nc.vector.tensor_mul(fl[:], fl[:], kf[:])
                    nc.vector.tensor_add(cidf[:, :, kk], fl[:], tof[:])
                nc.vector.tensor_scalar_add(cidf[:], cidf[:], -1.0)
                nc.vector.tensor_copy(cidx[:], cidf[:])
                if debug:
                    nc.sync.dma_start(dbg["cidx"][:], cidx[:])
                    nc.sync.dma_start(dbg["g12"][:], g12[:])

            # ======== P6: expert MLP; AG4 ========
            with tc.tile_pool(name="p6", bufs=1) as p6, \
                 tc.tile_pool(name="p6g", bufs=2) as p6g, \
                 tc.tile_pool(name="p6w", bufs=2) as p6w, \
                 tc.tile_pool(name="p6w2", bufs=2) as p6w2, \
                 tc.tile_pool(name="p6ps", bufs=2, space="PSUM") as p6ps, \
                 tc.tile_pool(name="p6pt", bufs=2, space="PSUM") as p6pt:
                bufT = p6.tile([128, 8, CAP], BF)
                for j in range(ST):
                    gb = p6g.tile([128, 1024], BF, tag="gb")
                    nc.vector.memset(gb[:], 0.0)
                    nc.gpsimd.indirect_dma_start(
                        out=gb[:], out_offset=None, in_=ag3_out[:],
                        in_offset=IndirectOffsetOnAxis(ap=gidx[:, j:j + 1], axis=0),
                        bounds_check=N - 1, oob_is_err=False)
                    for dc in range(8):
                        tp = p6pt.tile([128, 128], BF, tag="btp", bufs=1)
                        nc.tensor.transpose(tp[:], gb[:, 128 * dc:128 * dc + 128],
                                            identb_sb[:])
                        nc.vector.tensor_copy(bufT[:, dc, 128 * j:128 * j + 128], tp[:])
                h1T = p6.tile([128, 32, CAP], BF)
                w1v = w1l.rearrange("(o p) f -> p o f", p=128)
                for ft in range(32):
                    wt = p6w.tile([128, 8, 128], BF, tag="w1t")
                    nc.sync.dma_start(wt[:], w1v[:, :, 128 * ft:128 * ft + 128])
                    for cs, cw in [(0, 512), (512, 128)]:
                        hp = p6ps.tile([128, 512], FP, tag="hp", bufs=2)
                        for dk in range(8):
                            nc.tensor.matmul(hp[:, 0:cw], wt[:, dk],
                                             bufT[:, dk, cs:cs + cw],
                                             start=(dk == 0), stop=(dk == 7))
                        nc.scalar.activation(h1T[:, ft, cs:cs + cw], hp[:, 0:cw],
                                             AF.Gelu, bias=b1l_sb[:, ft:ft + 1])
                eo = p6.tile([128, ST, D], BF)
                w2v = w2l.rearrange("(o p) d -> p o d", p=128)
                for dn in range(2):
                    ops = [p6ps.tile([128, 512], FP, tag=f"op{st}", bufs=1,
                                     name=f"opst{st}")
                           for st in range(ST)]
                    for fkh in range(2):
                        w2t = p6w2.tile([128, 16, 512], BF, tag="w2t")
                        nc.sync.dma_start(w2t[:], w2v[:, 16 * fkh:16 * fkh + 16,
                                                      512 * dn:512 * dn + 512])
                        for st in range(ST):
                            for fi in range(16):
                                fk = 16 * fkh + fi
                                nc.tensor.matmul(ops[st][:],
                                                 h1T[:, fk, 128 * st:128 * st + 128],
                                                 w2t[:, fi], start=(fk == 0),
                                                 stop=(fk == 31))
                    for st in range(ST):
                        nc.vector.tensor_add(eo[:, st, 512 * dn:512 * dn + 512],
                                             ops[st][:],
                                             b2l_sb[:, 512 * dn:512 * dn + 512])
                nc.sync.dma_start(ag4_in.rearrange("(s p) d -> p s d", p=128), eo[:])
                if debug:
                    nc.sync.dma_start(dbg["eo"].rearrange("(s p) d -> p s d", p=128),
                                      eo[:])
            nc.gpsimd.collective_compute("AllGather", AL.bypass, RG,
                                         ins=[ag4_in.opt()], outs=[ag4_out.opt()])

            # ======== P7: combine ========
            with tc.tile_pool(name="p7", bufs=3) as p7:
                yv = y_out.rearrange("(t p) d -> p t d", p=128)
                for tt in range(4):
                    rows = []
                    for kk in range(2):
                        cr = p7.tile([128, D], BF, tag=f"cr{kk}")
                        nc.vector.memset(cr[:], 0.0)
                        nc.gpsimd.indirect_dma_start(
                            out=cr[:], out_offset=None, in_=ag4_out[:],
                            in_offset=IndirectOffsetOnAxis(ap=cidx[:, tt, kk:kk + 1],
                                                           axis=0),
                            bounds_check=NC * CAP - 1, oob_is_err=False)
                        rows.append(cr)
                    yt = p7.tile([128, D], FP, tag="yt")
                    nc.vector.scalar_tensor_tensor(yt[:], rows[0][:], g12[:, tt, 0:1],
                                                   h_sb[:, tt], AL.mult, AL.add)
                    nc.vector.scalar_tensor_tensor(yt[:], rows[1][:], g12[:, tt, 1:2],
                                                   yt[:], AL.mult, AL.add)
                    nc.sync.dma_start(yv[:, tt], yt[:])

    nc.compile()
    return nc


def _host_inputs(x, ln1_g, ln1_b, w_qkv, w_proj, ln2_g, ln2_b,
                 w_gate, w1, b1, w2, b2):
    x2d = np.asarray(x, np.float32).reshape(N, D)
    w_qkv = np.asarray(w_qkv, np.float32)
    w_proj = np.asarray(w_proj, np.float32)
    ln1_g = np.asarray(ln1_g, np.float32); ln1_b = np.asarray(ln1_b, np.float32)
    ln2_g = np.asarray(ln2_g, np.float32); ln2_b = np.asarray(ln2_b, np.float32)
    w_gate = np.asarray(w_gate, np.float32)
    w1 = np.asarray(w1, np.float32); b1 = np.asarray(b1, np.float32)
    w2 = np.asarray(w2, np.float32); b2 = np.asarray(b2, np.float32)

    pos = np.arange(T, dtype=np.float32)[:, None]
    inv = 1.0 / (10000.0 ** (np.arange(0, HD, 2, dtype=np.float32) / HD))
    ang = pos * inv
    sinN = np.tile(np.sin(ang).T, (1, B))
    cosN = np.tile(np.cos(ang).T, (1, B))
    sc = 1.0 / math.sqrt(HD)
    r128 = lambda a: np.ascontiguousarray(a.reshape(128, NT, 128))
    # fused q|k RoPE tables: rows 0:64 q (pre-scaled), 64:128 k
    Cqk_np = np.concatenate([cosN * sc, cosN * sc, cosN, cosN], 0)
    # 32-row blocks pre-swapped so each RoPE mul reads both inputs at the
    # same base partition (BIR SB-SB constraint)
    Sqk_np = np.concatenate([sinN * sc, -sinN * sc, sinN, -sinN], 0)

    c1 = (ln2_g[None, :] * w_gate).sum(1).astype(np.float32)
    c2 = (w_gate @ ln2_b).astype(np.float32)
    gate_fold = (ln2_g[None, :] * w_gate).astype(np.float32)
    # exact fp64 fold of the gate-logit attention path: w_gate·ln2_g @ w_proj
    g_proj = ((ln2_g[None, :] * w_gate).astype(np.float64)
              @ w_proj.astype(np.float64))  # [8, 1024]

    def ktiles(a):  # [1024, F] row-major (d = 128*o + p) -> [128, 8, F]
        return np.ascontiguousarray(
            a.reshape(8, 128, a.shape[1]).transpose(1, 0, 2))

    base = {
        "identb": np.eye(128, dtype=ml_dtypes.bfloat16),
        "identf": np.eye(128, dtype=np.float32),
        "tri": (np.arange(128)[:, None] <= np.arange(128)[None, :]).astype(ml_dtypes.bfloat16),
        "tri16": (np.arange(128)[:, None] <= np.arange(128)[None, :]).astype(np.float16),
        "m00": (((np.arange(128)[:, None] // 8) < (np.arange(128)[None, :] // 8))
                & ((np.arange(128)[:, None] % 8) == (np.arange(128)[None, :] % 8))
                ).astype(ml_dtypes.bfloat16),
        "m01": ((np.arange(128)[:, None] % 8) == (np.arange(128)[None, :] % 8)
                ).astype(ml_dtypes.bfloat16),
        "onesf": np.ones((128, 1), np.float32),
        "onesb": np.ones((128, 1), ml_dtypes.bfloat16),
        "ones_r_f": np.ones((1, 128), np.float32),
        "ln1g": np.ascontiguousarray(ln1_g.reshape(8, 128).T),
        "ln1b": np.ascontiguousarray(ln1_b.reshape(8, 128).T),
        "cc1": np.tile(c1, (128, 1)),
        "cc2": np.tile(c2, (128, 1)),
        "base8": np.tile(np.arange(8, dtype=np.float32) * CAP, (128, 1)),
        "Cqk": r128(Cqk_np).astype(NPATT),
        "Sqk": r128(Sqk_np).astype(NPATT),
        "tid1": np.ascontiguousarray(
            (np.arange(N, dtype=np.int32) + 1).reshape(NT, 128).T),
        "proj_w": ktiles(w_proj.T.copy()).astype(NPATT),
        "gate_w": ktiles(gate_fold.T.copy()),
    }

    in_maps = []
    for c in range(NC):
        h0, h1 = 2 * c, 2 * c + 1
        qs = lambda h: w_qkv[192 * h:192 * h + 64]
        ks = lambda h: w_qkv[192 * h + 64:192 * h + 128]
        vs = lambda h: w_qkv[192 * h + 128:192 * h + 192]
        qk = np.concatenate([qs(h0), ks(h0), qs(h1), ks(h1)], 0).T.copy()
        vw = np.concatenate([vs(h0), vs(h1)], 0).T.copy()
        shs = np.zeros((4, NT), np.float32)
        for tt in range(4):
            shs[tt, 4 * c + tt] = 1.0
        ese = np.zeros(8, np.float32)
        ese[c] = 1.0
        m = dict(base)
        m.update({
            "xT": np.ascontiguousarray(
                x2d[SH * c:SH * (c + 1)].T.reshape(8, 128, SH).transpose(1, 0, 2)),
            "qk_w": ktiles(qk).astype(NPATT),
            "v_w": ktiles(vw).astype(NPATT),
            "gpc": np.ascontiguousarray(
                g_proj[:, 128 * c:128 * (c + 1)].T.reshape(2, 64, 8)
                .transpose(1, 0, 2)).astype(np.float32),
            "esel": np.tile(ese, (128, 1)),
            "shsel": np.tile(shs[None], (128, 1, 1)),
            "w1l": (ln2_g[:, None] * w1[c]).astype(ml_dtypes.bfloat16),
            "b1l": np.ascontiguousarray(
                (b1[c] + ln2_b @ w1[c]).astype(np.float32).reshape(32, 128).T),
            "w2l": w2[c].astype(ml_dtypes.bfloat16),
            "b2l": np.tile(b2[c], (128, 1)).astype(np.float32),
        })
        in_maps.append(m)
    return in_maps


_NC_CACHE = {}


def _get_nc(debug=False):
    key = bool(debug)
    if key not in _NC_CACHE:
        _NC_CACHE[key] = build(debug=debug)
    return _NC_CACHE[key]


def kernel(**inputs):
    debug = bool(inputs.pop("_debug", False))
    want_results = inputs.pop("_want_results", False)
    trace = bool(inputs.pop("_trace", False))
    trace_cores = inputs.pop("_trace_cores", None)
    ncm = _get_nc(debug=debug)
    in_maps = _host_inputs(**inputs)
    kw = {}
    if trace_cores is not None:
        kw["trace_cores"] = trace_cores
    res = run_bass_kernel_spmd(ncm, in_maps, core_ids=list(range(NC)), trace=trace,
                               **kw)
    y = np.concatenate([res.results[c]["y"] for c in range(NC)], 0).reshape(B, T, D)
    if want_results:
        return y, res
    return y

